# revision 54
# baseline (speedup 1.0000x reference)
"""Trainium2 Bass kernel for the R-BERT-style MoE routing head.

Computes, for x [B, H]:
    binary_logits = tanh(x) @ W_bin + b_bin          # [B, 2]
    route         = argmax(binary_logits, axis=1)    # ties -> 0
    logits        = (x @ W0 + b0) if route==0 else (x @ W1 + b1)   # [B, 30]

Data-parallel over 8 NeuronCores: x is sharded along the batch axis; the tiny
head weights are replicated. Per core, rows are processed in groups of four
128-row tiles (one 2 MiB input DMA per group):

  1. DMA a natural [128, 4, 1024] x group into SBUF (contiguous, full-BW).
  2. TensorE transpose-mode matmuls flip each [128, 128] chunk into PSUM so
     the contraction dim (H) lands on partitions (these pipeline at ~110ns
     per block back-to-back).
  3. Per tile, one VectorE copy evicts the transposed tile to SBUF as
     float32r (expert operand) and one ScalarE tanh evicts it again in fp32
     as the binary-head operand — both into a group-interleaved
     [128, KC, 4, 128] layout so each matmul's moving operand covers the
     whole group contiguously (N=512).
  4. TensorE accumulates W.T @ xT (experts, float32r single-pass, PSUM rows
     0:64) and W_bin.T @ tanh(x)T (binary, full fp32 2-pass, rows 64:66)
     over the 8 K-chunks with the tiny weights stationary.  The binary head
     must stay fp32: the smallest routing margin over this input is 1.77e-5
     while float32r carries ~1e-4 error (would flip routes); its regular
     fp32 matmuls also keep the PE activity monitor warm (2.4 GHz) — f32r
     and transpose-mode work alone would leave the PE clock-gated at 1.2.
  5. One ScalarE Identity+bias op evicts the [66, 512] result, TensorE
     transposes each tile back to row-major, and VectorE selects expert 0
     vs 1 per row with a predicated copy (uint8 mask = bin1 > bin0). One
     packed [128, 4, 32] tile per group goes out on the scalar DMA ring.
"""

import numpy as np

B, H = 65536, 1024
NCORES = 8
BC = B // NCORES          # rows per core
PT = 128                  # rows per tile
KC = H // 128             # contraction chunks
NL = 30                   # labels per expert
NEXP = 2 * NL             # stacked expert outputs
NEXPP = 64                # expert outputs padded so psum rows 60:64 stay finite
BIN0 = 64                 # psum partition where binary logits start (32-aligned)
NROWS = BIN0 + 2          # psum rows: 60 expert + pad + 2 binary = 66
NOUT = NL + 2             # packed output cols: selected logits + binary
NA = 4                    # row-tiles per group (matmul moving N = NA*128 = 512)

_CACHE = {}


def _build(bc):
    import concourse.bacc as bacc
    import concourse.tile as tile
    from concourse import mybir

    f32 = mybir.dt.float32
    f32r = mybir.dt.float32r
    ngrp = bc // (NA * PT)

    nc = bacc.Bacc(
        "TRN2",
        target_bir_lowering=False,
        debug=False,
        enable_asserts=False,
        num_devices=NCORES,
    )

    xs = nc.dram_tensor("xs", [bc, H], f32, kind="ExternalInput")
    wexp = nc.dram_tensor("wexp", [128, KC, NEXPP], f32r, kind="ExternalInput")
    wbin = nc.dram_tensor("wbin", [128, KC, 2], f32, kind="ExternalInput")
    bias = nc.dram_tensor("bias", [NROWS, 1], f32, kind="ExternalInput")
    ident = nc.dram_tensor("ident", [128, 128], f32, kind="ExternalInput")
    out = nc.dram_tensor("out", [bc, NOUT], f32, kind="ExternalOutput")

    with tile.TileContext(nc) as tc:
        with (
            tc.tile_pool(name="const", bufs=1) as cpool,
            tc.tile_pool(name="x", bufs=4) as xpool,
            tc.tile_pool(name="xt", bufs=2) as xtpool,
            tc.tile_pool(name="tt", bufs=2) as ttpool,
            tc.tile_pool(name="ot", bufs=2) as opool,
            tc.tile_pool(name="fin", bufs=3) as fpool,
            tc.tile_pool(name="msk", bufs=3) as mpool,
            tc.tile_pool(name="pxt", bufs=3, space="PSUM") as pxt,
            tc.tile_pool(name="po", bufs=1, space="PSUM") as pout,
            tc.tile_pool(name="pn", bufs=1, space="PSUM") as pnat,
        ):
            # tiles allocated here; the weight/bias DMAs themselves are
            # emitted after the first x loads (they are not needed until the
            # first matmul phase, ~10us in)
            # identity rides the gpsimd (SWDGE) ring so it loads in parallel
            # with the first x tile on the sync ring
            ident_sb = cpool.tile([128, 128], f32)
            nc.gpsimd.dma_start(ident_sb[:], ident[:, :])
            wexp_sb = cpool.tile([128, KC, NEXPP], f32r)
            wbin_sb = cpool.tile([128, KC, 2], f32)
            bias_sb = cpool.tile([NROWS, 1], f32)

            def epilogue(psum_o, r0):
                # bias-add fused into the ACT eviction of psum_o
                outT = opool.tile([NROWS, NA, PT], f32, tag="ot")
                nc.scalar.activation(
                    outT[:],
                    psum_o[:].rearrange("p (a r) -> p a r", a=NA),
                    mybir.ActivationFunctionType.Identity,
                    bias=bias_sb[:, 0:1],
                )

                # back to row-major per tile: [66, 128] -> [128, 66]
                psum_n = pnat.tile([128, NA, NROWS], f32, tag="pn")
                for a in range(NA):
                    nc.tensor.transpose(
                        psum_n[:, a, :],
                        outT[:, a, :],
                        ident_sb[0:NROWS, 0:NROWS],
                    )

                fin = fpool.tile([PT, NA, NOUT], f32, tag="fin")
                nc.vector.tensor_copy(
                    fin[:, :, NL:NOUT], psum_n[:, :, BIN0 : BIN0 + 2]
                )
                # routing mask: 1 where bin1 > bin0 (argmax ties pick 0);
                # walrus requires an integer mask dtype for CopyPredicated
                mask = mpool.tile([PT, NA], mybir.dt.uint8, tag="msk")
                nc.vector.tensor_tensor(
                    mask[:].unsqueeze(2),
                    fin[:, :, NL + 1 : NL + 2],
                    fin[:, :, NL : NL + 1],
                    op=mybir.AluOpType.is_gt,
                )
                nc.vector.tensor_copy(fin[:, :, 0:NL], psum_n[:, :, 0:NL])
                # route==1 rows take expert-1 logits
                nc.vector.copy_predicated(
                    fin[:, :, 0:NL],
                    mask[:].unsqueeze(2).broadcast_to((PT, NA, NL)),
                    psum_n[:, :, NL:NEXP],
                )
                nc.scalar.dma_start(
                    out[r0 : r0 + NA * PT, :].rearrange("(a p) c -> p a c", p=PT),
                    fin[:],
                )

            pending = None  # previous group's (psum_o, r0), epilogue deferred
            for g in range(ngrp):
                r0 = g * NA * PT
                xg = xpool.tile([PT, NA, H], f32, tag="x")
                if g == 0:
                    # split the first load so compute starts after 512 KiB
                    for a in range(NA):
                        nc.sync.dma_start(
                            xg[:, a, :], xs[a * PT : (a + 1) * PT, :]
                        )
                    nc.sync.dma_start(wexp_sb[:], wexp[:, :, :])
                    nc.sync.dma_start(wbin_sb[:], wbin[:, :, :])
                    nc.sync.dma_start(bias_sb[:], bias[:, :])
                else:
                    nc.sync.dma_start(
                        xg[:],
                        xs[r0 : r0 + NA * PT, :].rearrange(
                            "(a p) h -> p a h", p=PT
                        ),
                    )

                # group-interleaved transposed operands: [h, k, a, rows]
                xT = xtpool.tile([128, KC, NA, PT], f32r, tag="xt")
                tT = ttpool.tile([128, KC, NA, PT], f32, tag="tt")
                for a in range(NA):
                    psum_xT = pxt.tile([128, H], f32, tag="pxt")
                    for k in range(KC):
                        sl = slice(k * 128, (k + 1) * 128)
                        nc.tensor.transpose(
                            psum_xT[:, sl], xg[:, a, sl], ident_sb[:]
                        )
                    src = psum_xT[:].rearrange("p (k r) -> p k r", k=KC)
                    if a < NA - 1:
                        nc.vector.tensor_copy(xT[:, :, a, :], src)
                    else:
                        # the last tile's expert operand gates the matmul
                        # phase: split its eviction across DVE and ACT so it
                        # lands ~2x sooner (ACT's tanh for this tile has
                        # slack — binary matmuls run after the expert wave)
                        half = KC // 2
                        nc.vector.tensor_copy(
                            xT[:, 0:half, a, :], src[:, 0:half, :]
                        )
                        nc.scalar.copy(
                            xT[:, half:KC, a, :], src[:, half:KC, :]
                        )
                    nc.scalar.activation(
                        tT[:, :, a, :], src, mybir.ActivationFunctionType.Tanh
                    )

                # previous group's epilogue lands here: its ACT eviction and
                # PE back-transposes hide under this group's transpose phase
                if pending is not None:
                    epilogue(*pending)

                psum_o = pout.tile([NROWS, NA * PT], f32, tag="po")
                for k in range(KC):
                    nc.tensor.matmul(
                        psum_o[0:NEXPP, :],
                        wexp_sb[:, k, :],
                        xT[:, k, :, :],
                        start=(k == 0),
                        stop=(k == KC - 1),
                        skip_group_check=True,
                    )
                for k in range(KC):
                    nc.tensor.matmul(
                        psum_o[BIN0 : BIN0 + 2, :],
                        wbin_sb[:, k, :],
                        tT[:, k, :, :],
                        start=(k == 0),
                        stop=(k == KC - 1),
                        skip_group_check=True,
                    )
                pending = (psum_o, r0)

            epilogue(*pending)

    nc.compile()
    return nc


def _get_nc(bc):
    key = ("nc", bc)
    if key not in _CACHE:
        _CACHE[key] = _build(bc)
    return _CACHE[key]


def _host_inputs(W_bin, b_bin, W0, b0, W1, b1):
    f32 = np.float32
    wall = np.concatenate(
        [
            np.asarray(W0, f32),
            np.asarray(W1, f32),
            np.zeros((H, NEXPP - NEXP), f32),
        ],
        axis=1,
    )  # [H, 64] — last 4 cols are zero padding
    wexp = np.ascontiguousarray(
        wall.reshape(KC, 128, NEXPP).transpose(1, 0, 2)
    )  # [128, KC, 64]
    wbin = np.ascontiguousarray(
        np.asarray(W_bin, f32).reshape(KC, 128, 2).transpose(1, 0, 2)
    )  # [128, KC, 2]
    bias = np.zeros((NROWS, 1), f32)
    bias[0:NL, 0] = np.asarray(b0, f32)
    bias[NL:NEXP, 0] = np.asarray(b1, f32)
    bias[BIN0 : BIN0 + 2, 0] = np.asarray(b_bin, f32)
    ident = np.eye(128, dtype=f32)
    return wexp, wbin, bias, ident


def _run(x, W_bin, b_bin, W0, b0, W1, b1, **spmd_kwargs):
    from concourse.bass_utils import run_bass_kernel_spmd

    x = np.ascontiguousarray(np.asarray(x, np.float32))
    wexp, wbin, bias, ident = _host_inputs(W_bin, b_bin, W0, b0, W1, b1)

    nc = _get_nc(BC)
    in_maps = [
        {
            "xs": x[c * BC : (c + 1) * BC],
            "wexp": wexp,
            "wbin": wbin,
            "bias": bias,
            "ident": ident,
        }
        for c in range(NCORES)
    ]
    res = run_bass_kernel_spmd(
        nc, in_maps, core_ids=list(range(NCORES)), **spmd_kwargs
    )
    full = np.concatenate([res.results[c]["out"] for c in range(NCORES)], axis=0)
    binary_logits = np.ascontiguousarray(full[:, NL:NOUT])
    logits = np.ascontiguousarray(full[:, 0:NL])
    return (binary_logits, logits), res


def kernel(x, W_bin, b_bin, W0, b0, W1, b1):
    outs, _ = _run(x, W_bin, b_bin, W0, b0, W1, b1)
    return outs


# revision 59
# speedup vs baseline: 1.0098x; 1.0098x over previous
"""Trainium2 Bass kernel for the R-BERT-style MoE routing head.

Computes, for x [B, H]:
    binary_logits = tanh(x) @ W_bin + b_bin          # [B, 2]
    route         = argmax(binary_logits, axis=1)    # ties -> 0
    logits        = (x @ W0 + b0) if route==0 else (x @ W1 + b1)   # [B, 30]

Data-parallel over 8 NeuronCores: x is sharded along the batch axis; the tiny
head weights are replicated. Per core, rows are processed in groups of four
128-row tiles (one 2 MiB input DMA per group):

  1. DMA a natural [128, 4, 1024] x group into SBUF (contiguous, full-BW).
  2. TensorE transpose-mode matmuls flip each [128, 128] chunk into PSUM so
     the contraction dim (H) lands on partitions (these pipeline at ~110ns
     per block back-to-back).
  3. Per tile, one VectorE copy evicts the transposed tile to SBUF as
     float32r (expert operand) and one ScalarE tanh evicts it again in fp32
     as the binary-head operand — both into a group-interleaved
     [128, KC, 4, 128] layout so each matmul's moving operand covers the
     whole group contiguously (N=512).
  4. TensorE accumulates W.T @ xT (experts, float32r single-pass, PSUM rows
     0:64) and W_bin.T @ tanh(x)T (binary, full fp32 2-pass, rows 64:66)
     over the 8 K-chunks with the tiny weights stationary.  The binary head
     must stay fp32: the smallest routing margin over this input is 1.77e-5
     while float32r carries ~1e-4 error (would flip routes); its regular
     fp32 matmuls also keep the PE activity monitor warm (2.4 GHz) — f32r
     and transpose-mode work alone would leave the PE clock-gated at 1.2.
  5. One ScalarE Identity+bias op evicts the [66, 512] result, TensorE
     transposes each tile back to row-major, and VectorE selects expert 0
     vs 1 per row with a predicated copy (uint8 mask = bin1 > bin0). One
     packed [128, 4, 32] tile per group goes out on the scalar DMA ring.
"""

import numpy as np

B, H = 65536, 1024
NCORES = 8
BC = B // NCORES          # rows per core
PT = 128                  # rows per tile
KC = H // 128             # contraction chunks
NL = 30                   # labels per expert
NEXP = 2 * NL             # stacked expert outputs
NEXPP = 64                # expert outputs padded so psum rows 60:64 stay finite
BIN0 = 64                 # psum partition where binary logits start (32-aligned)
NROWS = BIN0 + 2          # psum rows: 60 expert + pad + 2 binary = 66
NOUT = NL + 2             # packed output cols: selected logits + binary
NA = 4                    # row-tiles per group (matmul moving N = NA*128 = 512)

_CACHE = {}


def _build(bc):
    import concourse.bacc as bacc
    import concourse.tile as tile
    from concourse import masks, mybir

    f32 = mybir.dt.float32
    f32r = mybir.dt.float32r
    ngrp = bc // (NA * PT)

    nc = bacc.Bacc(
        "TRN2",
        target_bir_lowering=False,
        debug=False,
        enable_asserts=False,
        num_devices=NCORES,
    )

    xs = nc.dram_tensor("xs", [bc, H], f32, kind="ExternalInput")
    wexp = nc.dram_tensor("wexp", [128, KC, NEXPP], f32r, kind="ExternalInput")
    wbin = nc.dram_tensor("wbin", [128, KC, 2], f32, kind="ExternalInput")
    bias = nc.dram_tensor("bias", [NROWS, 1], f32, kind="ExternalInput")
    out = nc.dram_tensor("out", [bc, NOUT], f32, kind="ExternalOutput")

    with tile.TileContext(nc) as tc:
        with (
            tc.tile_pool(name="const", bufs=1) as cpool,
            tc.tile_pool(name="x", bufs=4) as xpool,
            tc.tile_pool(name="xt", bufs=2) as xtpool,
            tc.tile_pool(name="tt", bufs=2) as ttpool,
            tc.tile_pool(name="ot", bufs=2) as opool,
            tc.tile_pool(name="fin", bufs=3) as fpool,
            tc.tile_pool(name="msk", bufs=3) as mpool,
            tc.tile_pool(name="pxt", bufs=3, space="PSUM") as pxt,
            tc.tile_pool(name="po", bufs=1, space="PSUM") as pout,
            tc.tile_pool(name="pn", bufs=1, space="PSUM") as pnat,
        ):
            # tiles allocated here; the weight/bias DMAs themselves are
            # emitted after the first x loads (they are not needed until the
            # first matmul phase, ~10us in)
            # identity built on the idle GPSIMD engine, in parallel with the
            # first x-tile DMA (no input DMA dependency at all)
            ident_sb = cpool.tile([128, 128], f32)
            masks.make_identity(nc, ident_sb[:])
            wexp_sb = cpool.tile([128, KC, NEXPP], f32r)
            wbin_sb = cpool.tile([128, KC, 2], f32)
            bias_sb = cpool.tile([NROWS, 1], f32)

            def epilogue(psum_o, r0):
                # bias-add fused into the ACT eviction of psum_o
                outT = opool.tile([NROWS, NA, PT], f32, tag="ot")
                nc.scalar.activation(
                    outT[:],
                    psum_o[:].rearrange("p (a r) -> p a r", a=NA),
                    mybir.ActivationFunctionType.Identity,
                    bias=bias_sb[:, 0:1],
                )

                # back to row-major per tile: [66, 128] -> [128, 66]
                psum_n = pnat.tile([128, NA, NROWS], f32, tag="pn")
                for a in range(NA):
                    nc.tensor.transpose(
                        psum_n[:, a, :],
                        outT[:, a, :],
                        ident_sb[0:NROWS, 0:NROWS],
                    )

                fin = fpool.tile([PT, NA, NOUT], f32, tag="fin")
                nc.vector.tensor_copy(
                    fin[:, :, NL:NOUT], psum_n[:, :, BIN0 : BIN0 + 2]
                )
                # routing mask: 1 where bin1 > bin0 (argmax ties pick 0);
                # walrus requires an integer mask dtype for CopyPredicated
                mask = mpool.tile([PT, NA], mybir.dt.uint8, tag="msk")
                nc.vector.tensor_tensor(
                    mask[:].unsqueeze(2),
                    fin[:, :, NL + 1 : NL + 2],
                    fin[:, :, NL : NL + 1],
                    op=mybir.AluOpType.is_gt,
                )
                nc.vector.tensor_copy(fin[:, :, 0:NL], psum_n[:, :, 0:NL])
                # route==1 rows take expert-1 logits
                nc.vector.copy_predicated(
                    fin[:, :, 0:NL],
                    mask[:].unsqueeze(2).broadcast_to((PT, NA, NL)),
                    psum_n[:, :, NL:NEXP],
                )
                nc.scalar.dma_start(
                    out[r0 : r0 + NA * PT, :].rearrange("(a p) c -> p a c", p=PT),
                    fin[:],
                )

            pending = None  # previous group's (psum_o, r0), epilogue deferred
            for g in range(ngrp):
                r0 = g * NA * PT
                xg = xpool.tile([PT, NA, H], f32, tag="x")
                if g == 0:
                    # split the first load so compute starts after 512 KiB
                    for a in range(NA):
                        nc.sync.dma_start(
                            xg[:, a, :], xs[a * PT : (a + 1) * PT, :]
                        )
                    nc.sync.dma_start(wexp_sb[:], wexp[:, :, :])
                    nc.sync.dma_start(wbin_sb[:], wbin[:, :, :])
                    nc.sync.dma_start(bias_sb[:], bias[:, :])
                else:
                    nc.sync.dma_start(
                        xg[:],
                        xs[r0 : r0 + NA * PT, :].rearrange(
                            "(a p) h -> p a h", p=PT
                        ),
                    )

                # group-interleaved transposed operands: [h, k, a, rows]
                xT = xtpool.tile([128, KC, NA, PT], f32r, tag="xt")
                tT = ttpool.tile([128, KC, NA, PT], f32, tag="tt")
                for a in range(NA):
                    psum_xT = pxt.tile([128, H], f32, tag="pxt")
                    for k in range(KC):
                        sl = slice(k * 128, (k + 1) * 128)
                        nc.tensor.transpose(
                            psum_xT[:, sl], xg[:, a, sl], ident_sb[:]
                        )
                    src = psum_xT[:].rearrange("p (k r) -> p k r", k=KC)
                    if a < NA - 1:
                        nc.vector.tensor_copy(xT[:, :, a, :], src)
                    else:
                        # the last tile's expert operand gates the matmul
                        # phase: split its eviction across DVE and ACT so it
                        # lands ~2x sooner (ACT's tanh for this tile has
                        # slack — binary matmuls run after the expert wave)
                        half = KC // 2
                        nc.vector.tensor_copy(
                            xT[:, 0:half, a, :], src[:, 0:half, :]
                        )
                        nc.scalar.copy(
                            xT[:, half:KC, a, :], src[:, half:KC, :]
                        )
                    nc.scalar.activation(
                        tT[:, :, a, :], src, mybir.ActivationFunctionType.Tanh
                    )

                # previous group's epilogue lands here: its ACT eviction and
                # PE back-transposes hide under this group's transpose phase
                if pending is not None:
                    epilogue(*pending)

                psum_o = pout.tile([NROWS, NA * PT], f32, tag="po")
                for k in range(KC):
                    nc.tensor.matmul(
                        psum_o[0:NEXPP, :],
                        wexp_sb[:, k, :],
                        xT[:, k, :, :],
                        start=(k == 0),
                        stop=(k == KC - 1),
                        skip_group_check=True,
                    )
                for k in range(KC):
                    nc.tensor.matmul(
                        psum_o[BIN0 : BIN0 + 2, :],
                        wbin_sb[:, k, :],
                        tT[:, k, :, :],
                        start=(k == 0),
                        stop=(k == KC - 1),
                        skip_group_check=True,
                    )
                pending = (psum_o, r0)

            epilogue(*pending)

    nc.compile()
    return nc


def _get_nc(bc):
    key = ("nc", bc)
    if key not in _CACHE:
        _CACHE[key] = _build(bc)
    return _CACHE[key]


def _host_inputs(W_bin, b_bin, W0, b0, W1, b1):
    f32 = np.float32
    wall = np.concatenate(
        [
            np.asarray(W0, f32),
            np.asarray(W1, f32),
            np.zeros((H, NEXPP - NEXP), f32),
        ],
        axis=1,
    )  # [H, 64] — last 4 cols are zero padding
    wexp = np.ascontiguousarray(
        wall.reshape(KC, 128, NEXPP).transpose(1, 0, 2)
    )  # [128, KC, 64]
    wbin = np.ascontiguousarray(
        np.asarray(W_bin, f32).reshape(KC, 128, 2).transpose(1, 0, 2)
    )  # [128, KC, 2]
    bias = np.zeros((NROWS, 1), f32)
    bias[0:NL, 0] = np.asarray(b0, f32)
    bias[NL:NEXP, 0] = np.asarray(b1, f32)
    bias[BIN0 : BIN0 + 2, 0] = np.asarray(b_bin, f32)
    ident = np.eye(128, dtype=f32)
    return wexp, wbin, bias, ident


def _run(x, W_bin, b_bin, W0, b0, W1, b1, **spmd_kwargs):
    from concourse.bass_utils import run_bass_kernel_spmd

    x = np.ascontiguousarray(np.asarray(x, np.float32))
    wexp, wbin, bias, ident = _host_inputs(W_bin, b_bin, W0, b0, W1, b1)

    nc = _get_nc(BC)
    in_maps = [
        {
            "xs": x[c * BC : (c + 1) * BC],
            "wexp": wexp,
            "wbin": wbin,
            "bias": bias,
        }
        for c in range(NCORES)
    ]
    res = run_bass_kernel_spmd(
        nc, in_maps, core_ids=list(range(NCORES)), **spmd_kwargs
    )
    full = np.concatenate([res.results[c]["out"] for c in range(NCORES)], axis=0)
    binary_logits = np.ascontiguousarray(full[:, NL:NOUT])
    logits = np.ascontiguousarray(full[:, 0:NL])
    return (binary_logits, logits), res


def kernel(x, W_bin, b_bin, W0, b0, W1, b1):
    outs, _ = _run(x, W_bin, b_bin, W0, b0, W1, b1)
    return outs
